# revision 10
# baseline (speedup 1.0000x reference)
"""Trainium2 Bass kernel for nn_Decoder: embedding -> masked LSTM(relu) -> BN ->
Dense(relu) -> vocab projection -> softmax. 8 NeuronCores, SPMD.

8-way model-parallel LSTM recurrence with per-step AllGather.

Each core owns u-chunk k (its core index): computes gates i,f,cc,o for its 128
u's, updates local c/h state, all-gathers the bf16 h shard so every core has
the full h.T [128, KC, B] for the next step's matmul. Wh/Wx are column-sharded
on host (per-core data, identical program). Phase C identical to v0
(token-sharded, local softmax).
"""
import sys
import contextlib

import numpy as np

sys.path.insert(0, "/opt/trn_rl_repo")

import ml_dtypes
import concourse.bass as bass
import concourse.tile as tile
from concourse import bacc
from concourse import mybir
from concourse.bass_utils import run_bass_kernel_spmd

N_CORES = 8
V, E, U, B, S = 32000, 512, 1024, 16, 128
BN_EPS = 1e-3
KC = U // 128
EC = E // 128
NB = B // N_CORES
NV = 500
NVC = V // NV
GW = 4 * 128           # per-core gate-col shard width (4 chunks: i,f,cc,o)
N_FILL = 24            # PE-warming filler matmuls per step

F32 = mybir.dt.float32
BF16 = mybir.dt.bfloat16
SIG = mybir.ActivationFunctionType.Sigmoid
REL = mybir.ActivationFunctionType.Relu
IDN = mybir.ActivationFunctionType.Identity
EXP = mybir.ActivationFunctionType.Exp

_CACHE = {}


def build():
    nc = bacc.Bacc(None, num_devices=N_CORES)

    xT = nc.declare_dram_parameter("xT", [E, B * S], F32, isOutput=False)
    U8 = mybir.dt.uint8
    mb = nc.declare_dram_parameter("mb", [128, B * S], U8, isOutput=False)
    h0 = nc.declare_dram_parameter("h0", [128, KC * B], F32, isOutput=False)
    h0o = nc.declare_dram_parameter("h0o", [128, B], F32, isOutput=False)
    c0o = nc.declare_dram_parameter("c0o", [128, B], F32, isOutput=False)
    wxs_p = nc.declare_dram_parameter("wxs", [E, GW], F32, isOutput=False)
    bxs_p = nc.declare_dram_parameter("bxs", [128, 4], F32, isOutput=False)
    whs_p = nc.declare_dram_parameter("whs", [U, GW], BF16, isOutput=False)
    w1 = nc.declare_dram_parameter("w1", [U, U], BF16, isOutput=False)
    b1 = nc.declare_dram_parameter("b1", [128, KC], F32, isOutput=False)
    bnsc = nc.declare_dram_parameter("bnsc", [128, KC], F32, isOutput=False)
    bnsh = nc.declare_dram_parameter("bnsh", [128, KC], F32, isOutput=False)
    w2 = nc.declare_dram_parameter("w2", [U, V], BF16, isOutput=False)
    b2 = nc.declare_dram_parameter("b2", [1, V], BF16, isOutput=False)
    out = nc.declare_dram_parameter("out", [NB, S, V], F32, isOutput=True)

    rg = [list(range(N_CORES))]

    with tile.TileContext(nc) as tc:
        ctx = contextlib.ExitStack()
        singles = ctx.enter_context(tc.tile_pool(name="singles", bufs=1))
        dram = ctx.enter_context(tc.tile_pool(name="dram", bufs=1, space="DRAM"))

        b1s = singles.tile([128, KC], F32)
        nc.sync.dma_start(b1s, b1[:, :])
        bnscs = singles.tile([128, KC], F32)
        nc.sync.dma_start(bnscs, bnsc[:, :])
        bnshs = singles.tile([128, KC], F32)
        nc.sync.dma_start(bnshs, bnsh[:, :])
        hsall = singles.tile([128, KC, S, NB], BF16)
        d1T = singles.tile([128, NB, KC, S], BF16)
        sums = singles.tile([128, NB, NVC], F32)
        ones1 = singles.tile([1, 128], BF16)
        nc.vector.memset(ones1, 1.0)

        # ============ phase A: xgT shard = wxs.T @ xT + bxs ============
        ctxAB = contextlib.ExitStack()
        pAB = ctxAB.enter_context(tc.tile_pool(name="pAB", bufs=1))
        xgts = pAB.tile([128, 4, B * S], F32)          # 32KB/part
        with (
            tc.tile_pool(name="pA", bufs=1) as pA,
            tc.tile_pool(name="psA", bufs=2, space="PSUM") as psA,
        ):
            xts = pA.tile([128, EC, B * S], F32)
            nc.sync.dma_start(xts, xT[:, :].rearrange("(k p) t -> p k t", p=128))
            wxss = pA.tile([128, EC, GW], F32)
            nc.sync.dma_start(wxss, wxs_p[:, :].rearrange("(k p) m -> p k m", p=128))
            bxss = pA.tile([128, 4], F32)
            nc.sync.dma_start(bxss, bxs_p[:, :])
            NT = B * S // 512
            for m in range(4):
                for n in range(NT):
                    ps = psA.tile([128, 512], F32, tag="psA")
                    for k in range(EC):
                        nc.tensor.matmul(
                            ps,
                            wxss[:, k, m * 128:(m + 1) * 128],
                            xts[:, k, n * 512:(n + 1) * 512],
                            start=(k == 0),
                            stop=(k == EC - 1),
                        )
                    nc.scalar.add(
                        xgts[:, m, n * 512:(n + 1) * 512], ps, bxss[:, m:m + 1]
                    )

        # ============ phase B: sharded recurrence with per-step AllGather ============
        with (
            tc.tile_pool(name="pB", bufs=1) as pB,
            tc.tile_pool(name="pBh", bufs=1) as pBh,
            tc.tile_pool(name="pBw", bufs=3) as pBw,
            tc.tile_pool(name="pBd", bufs=1, space="DRAM") as pBd,
            tc.tile_pool(name="psB", bufs=2, space="PSUM") as psB,
        ):
            whss = pB.tile([128, KC, GW], BF16)        # 4KB/part
            nc.sync.dma_start(whss, whs_p[:, :].rearrange("(k p) m -> p k m", p=128))
            mbs = pB.tile([128, B * S], U8)
            nc.sync.dma_start(mbs, mb[:, :])
            h_own_f = pB.tile([128, B], F32)
            nc.sync.dma_start(h_own_f, h0o[:, :])
            h_own = pB.tile([128, B], BF16)
            nc.vector.tensor_copy(h_own, h_own_f)
            c_own = pB.tile([128, B], F32)
            nc.sync.dma_start(c_own, c0o[:, :])
            pid = nc.partition_id()
            b0 = pid * NB

            hg_prev = pBh.tile([128, KC, B], BF16, tag="hg_init")
            hg_f32 = pBw.tile([128, KC, B], F32, tag="hg0f")
            nc.sync.dma_start(hg_f32, h0[:, :].rearrange("p (k b) -> p k b", k=KC))
            nc.vector.tensor_copy(hg_prev, hg_f32)

            for s in range(S):
                zt = psB.tile([128, 4, B], F32, tag="zt")
                for m in range(4):
                    for k in range(KC):
                        nc.tensor.matmul(
                            zt[:, m, :],
                            whss[:, k, m * 128:(m + 1) * 128],
                            hg_prev[:, k, :],
                            start=(k == 0),
                            stop=(k == KC - 1),
                        )
                zsb = pBw.tile([128, 4, B], F32, tag="zsb")
                nc.vector.tensor_add(
                    zsb, zt, xgts[:, :, s * B:(s + 1) * B]
                )
                nc.scalar.activation(zsb[:, 0:2, :], zsb[:, 0:2, :], SIG)
                nc.scalar.activation(zsb[:, 2:3, :], zsb[:, 2:3, :], REL)
                nc.scalar.activation(zsb[:, 3:4, :], zsb[:, 3:4, :], SIG)
                t1 = pBw.tile([128, B], F32, tag="t1")
                nc.vector.tensor_mul(t1, zsb[:, 1, :], c_own)
                t2 = pBw.tile([128, B], F32, tag="t2")
                nc.vector.tensor_mul(t2, zsb[:, 0, :], zsb[:, 2, :])
                nc.vector.tensor_add(t1, t1, t2)
                msl = mbs[:, s * B:(s + 1) * B]
                c_new = pBw.tile([128, B], F32, tag="cT")
                nc.vector.tensor_copy(c_new, c_own)
                nc.vector.copy_predicated(c_new, msl, t1)
                nc.scalar.activation(t2, c_new, REL)
                nc.vector.tensor_mul(t2, zsb[:, 3, :], t2)
                h_new = pBw.tile([128, B], BF16, tag="hT")
                nc.vector.tensor_copy(h_new, h_own)
                nc.vector.copy_predicated(h_new, msl, t2)
                agin = pBd.tile([128, B], BF16, tag=f"agin{s}")
                nc.sync.dma_start(agin[:, :], h_new)
                agout = pBd.tile([KC * 128, B], BF16, tag=f"agout{s}")
                nc.gpsimd.collective_compute(
                    "AllGather",
                    mybir.AluOpType.bypass,
                    replica_groups=rg,
                    ins=[agin[:, :]],
                    outs=[agout[:, :]],
                )
                hg = pBh.tile([128, KC, B], BF16, tag=f"hg{s}")
                nc.sync.dma_start(
                    hg, agout[:, :].rearrange("(k p) b -> p k b", p=128)
                )
                nc.vector.tensor_copy(
                    hsall[:, :, s, :], hg[:, :, bass.ds(b0, NB)]
                )
                h_own, c_own = h_new, c_new
                hg_prev = hg

        ctxAB.close()

        # ============ phase C: token-sharded (identical to v0) ============
        with (
            tc.tile_pool(name="pC", bufs=1) as pC,
            tc.tile_pool(name="pCw", bufs=3) as pCw,
            tc.tile_pool(name="pCw2", bufs=2) as pCw2,
            tc.tile_pool(name="psC", bufs=2, space="PSUM") as psC,
            tc.tile_pool(name="psL", bufs=2, space="PSUM") as psL,
        ):
            w1s = pC.tile([128, KC, U], BF16)
            nc.sync.dma_start(w1s, w1[:, :].rearrange("(k p) m -> p k m", p=128))
            F16 = mybir.dt.float16
            expb = pC.tile([128, NB, NVC, NV], F16)   # 125KB/part
            for b in range(NB):
                bnh = pCw.tile([128, KC, S], BF16, tag="bnh")
                for k in range(KC):
                    nc.scalar.activation(
                        bnh[:, k, :], hsall[:, k, :, b], IDN,
                        bias=bnshs[:, k:k + 1], scale=bnscs[:, k:k + 1],
                    )
                for mo in range(KC):
                    dps = psC.tile([128, S], F32, tag="dps")
                    for k in range(KC):
                        nc.tensor.matmul(
                            dps,
                            w1s[:, k, mo * 128:(mo + 1) * 128],
                            bnh[:, k, :],
                            start=(k == 0),
                            stop=(k == KC - 1),
                        )
                    nc.scalar.activation(
                        d1T[:, b, mo, :], dps, REL, bias=b1s[:, mo:mo + 1]
                    )

            for n in range(NVC):
                w2t = pCw2.tile([128, KC, NV], BF16, tag="w2t")
                nc.sync.dma_start(
                    w2t,
                    w2[:, n * NV:(n + 1) * NV].rearrange("(k p) v -> p k v", p=128),
                )
                b2t = pCw2.tile([1, NV], BF16, tag="b2t")
                nc.sync.dma_start(b2t, b2[0:1, n * NV:(n + 1) * NV])
                for b in range(NB):
                    lg = psL.tile([128, NV], F32, tag="lg")
                    for k in range(KC):
                        nc.tensor.matmul(
                            lg, d1T[:, b, k, :], w2t[:, k, :],
                            start=(k == 0), stop=False,
                        )
                    nc.tensor.matmul(lg, ones1, b2t, start=False, stop=True)
                    nc.scalar.activation(
                        expb[:, b, n, :], lg, EXP, accum_out=sums[:, b, n:n + 1]
                    )

            gsum = pC.tile([128, NB], F32)
            nc.vector.tensor_reduce(
                gsum, sums, mybir.AxisListType.X, mybir.AluOpType.add
            )
            grecip = pC.tile([128, NB], F32)
            nc.vector.reciprocal(grecip, gsum)
            for b in range(NB):
                for n in range(NVC):
                    ot = pCw.tile([128, NV], F32, tag="ot")
                    nc.vector.tensor_scalar_mul(
                        ot, expb[:, b, n, :], grecip[:, b:b + 1]
                    )
                    nc.sync.dma_start(out[b, :, n * NV:(n + 1) * NV], ot)
        ctx.close()
    nc.finalize()
    return nc


def _prep_inputs(inputs):
    inputs = {k: np.asarray(v) for k, v in inputs.items()}
    idx = inputs["inputs"].astype(np.int64)
    emb = inputs["emb"].astype(np.float32)
    x = emb[idx]
    mask = (idx != 0).astype(np.float32)

    Wx = inputs["Wx"].astype(np.float32)
    Wh = inputs["Wh"]
    W116 = inputs["W1"].astype(ml_dtypes.bfloat16)
    W216 = inputs["W2"].astype(ml_dtypes.bfloat16)
    b_ = inputs["b"].astype(np.float32)
    b1_ = inputs["b1"].astype(np.float32)
    b2_ = np.ascontiguousarray(inputs["b2"].reshape(1, V).astype(ml_dtypes.bfloat16))
    sc = (inputs["gamma"] / np.sqrt(inputs["mov_var"] + BN_EPS)).astype(np.float32)
    sh = (inputs["beta"] - inputs["mov_mean"] * sc).astype(np.float32)

    b1c = np.ascontiguousarray(b1_.reshape(KC, 128).T)
    scc = np.ascontiguousarray(sc.reshape(KC, 128).T)
    shc = np.ascontiguousarray(sh.reshape(KC, 128).T)

    h0f = inputs["enc_hidden"].astype(np.float32)
    c0f = inputs["enc_cell"].astype(np.float32)

    in_maps = []
    for c in range(N_CORES):
        perm = list(range(B))  # canonical order (required: AG mixes cores' shards)
        xp = x
        xTc = np.ascontiguousarray(xp.transpose(2, 1, 0).reshape(E, S * B), np.float32)  # t = s*B + b
        mbc = np.ascontiguousarray(
            np.broadcast_to(mask[perm].T.reshape(1, S * B), (128, S * B)), np.uint8
        )
        h0c = np.ascontiguousarray(
            h0f[perm].reshape(B, KC, 128).transpose(2, 1, 0).reshape(128, KC * B)
        )
        # own u-chunk = c (u rows 128c..128c+127)
        h0oc = np.ascontiguousarray(h0f[perm][:, c * 128:(c + 1) * 128].T)
        c0oc = np.ascontiguousarray(c0f[perm][:, c * 128:(c + 1) * 128].T)
        # gate-col shard: for gates g=0..3 pick cols g*U + [128c, 128c+128)
        cols = np.concatenate(
            [np.arange(g * U + c * 128, g * U + (c + 1) * 128) for g in range(4)]
        )
        wxsh = np.ascontiguousarray(Wx[:, cols])
        whsh = np.ascontiguousarray(Wh[:, cols]).astype(ml_dtypes.bfloat16)
        bxsh = np.ascontiguousarray(b_[cols].reshape(4, 128).T)
        in_maps.append({
            "xT": xTc, "mb": mbc, "h0": h0c, "h0o": h0oc, "c0o": c0oc,
            "wxs": wxsh, "bxs": bxsh, "whs": whsh,
            "w1": W116, "b1": b1c, "bnsc": scc, "bnsh": shc,
            "w2": W216, "b2": b2_,
        })
    return in_maps


def kernel(**inputs) -> np.ndarray:
    if "nc" not in _CACHE:
        _CACHE["nc"] = build()
    nc = _CACHE["nc"]
    in_maps = _prep_inputs(inputs)
    res = run_bass_kernel_spmd(nc, in_maps, list(range(N_CORES)))
    outs = [res.results[c]["out"] for c in range(N_CORES)]
    full = np.concatenate(outs, axis=0)
    return np.ascontiguousarray(full, np.float32)
